# revision 3
# baseline (speedup 1.0000x reference)
"""Trainium2 Bass kernel for nn_ClusterModel (MoE routing + segment pooling).

Model:
  xg = x[group_indices]                         # [4, N/4, 128] per-group gather
  h  = relu(xg @ W1[g] + b1[g])                 # [4, N/4, 1024]
  og = h @ W2[g] + b2[g]                        # [4, N/4, 512]
  new_feat = scatter(og) back to node order     # [N, 512]
  emb = segment_max(new_feat, fine clusters)    # [8192, 512]  (16 nodes/cluster)
  normed = InstanceNorm per coarse graph        # [8192, 512]  (256 clusters/graph)
  logits = normed @ w_out + b_out               # [8192, 16]

Sharding: 8 cores, each takes 4 consecutive coarse graphs (= 16384 nodes =
1024 fine clusters).  All segment reductions are core-local -> no collectives.

Per-core pipeline (graph-major, all engines overlapped):
  for G in 4 graphs:
    for g in 4 groups:  bf16 GEMM over that graph's group-g rows
      L1 (W1 stationary, xt moving)  -> h feature-major in PSUM
      relu+b1 (ACT/DVE alternating)  -> ht bf16
      L2 (ht stationary, W2 moving)  -> og row-major [128 rows, 512] in PSUM
      +b2 (DVE) -> og bf16 -> DMA to per-graph DRAM scratch
    2x dma_gather (non-transpose, 1KB row descriptors) -> [128cl, 16m, 512]
    DVE pairwise max tree over members -> emb half [128, 512] bf16
    PE transpose -> feature-major embT; InstanceNorm stats; classifier MM.

GEMM inputs bf16 (full PE rate, FWL weight loads); stats/classifier f32/f32r.
Graph G's gather/pool/norm overlaps graph G+1's GEMM; PE stays HAM-warm.
"""

import numpy as np
from contextlib import ExitStack

import jax
import concourse.bass as bass
import concourse.tile as tile
from concourse import bacc, mybir
from concourse import bass2jax

F32 = mybir.dt.float32
F32R = mybir.dt.float32r
BF16 = mybir.dt.bfloat16
I16 = mybir.dt.int16
AF = mybir.ActivationFunctionType
ALU = mybir.AluOpType
NPBF16 = mybir.dt.np(BF16)

# Problem constants (hardcoded per contest contract)
N = 131072
D = 128
KEXP = 1024
H = 512
NG = 4
F_SEG = 8192
G_SEG = 32
C_CLS = 16
EPS = 1e-5
NCORES = 8
P = 128
KT = KEXP // P                 # 8 k-tiles in layer 2
FT = H // P                    # 4 feature tiles
NEG = -3.0e38

_PROGRAM_CACHE: dict = {}


# ----------------------------------------------------------------------------
# Device program
# ----------------------------------------------------------------------------

def _build_program(K_CAP: int, CCAP: int, MCAP: int):
    """SPMD program. K_CAP: padded rows per (graph, group), multiple of 128.
    CCAP: padded clusters per graph, multiple of 128. MCAP: padded members
    per cluster, power of two."""
    GPC = G_SEG // NCORES          # graphs per core = 4
    NHALF = CCAP // P              # gather blocks per graph (128 clusters each)
    RTOT = GPC * NG * K_CAP        # xt columns per core
    RT_G = NG * K_CAP + 1          # og rows per graph (+1 zero row)
    ZROW = NG * K_CAP
    SLOTS = GPC * CCAP
    IW = MCAP * P // 16            # idx words per gather block

    # chunk split of K_CAP into <=512 pieces (multiples of 128)
    chunks = []
    off = 0
    while off < K_CAP:
        n = min(512, K_CAP - off)
        chunks.append((off, n))
        off += n

    nc = bacc.Bacc("TRN2", target_bir_lowering=False, debug=False,
                   num_devices=NCORES)

    xt_ap = nc.dram_tensor("xt", [P, RTOT], BF16, kind="ExternalInput").ap()
    w1_ap = nc.dram_tensor("w1b", [P, NG, KEXP], BF16, kind="ExternalInput").ap()
    w2_ap = nc.dram_tensor("w2b", [P, NG, KT, H], BF16, kind="ExternalInput").ap()
    b1_ap = nc.dram_tensor("b1s", [P, NG * KT], F32, kind="ExternalInput").ap()
    b2_ap = nc.dram_tensor("b2r", [P, NG, H], F32, kind="ExternalInput").ap()
    wo_ap = nc.dram_tensor("wout", [P, FT, C_CLS], F32, kind="ExternalInput").ap()
    bo_ap = nc.dram_tensor("bout", [C_CLS, 1], F32, kind="ExternalInput").ap()
    ic_ap = nc.dram_tensor("invc", [P, GPC], F32, kind="ExternalInput").ap()
    gi_ap = nc.dram_tensor("gidx", [P, GPC * NHALF * IW], I16,
                           kind="ExternalInput").ap()
    id_ap = nc.dram_tensor("ident", [P, P], BF16, kind="ExternalInput").ap()
    og_aps = [nc.dram_tensor(f"ogs{G}", [RT_G, H], BF16).ap()
              for G in range(GPC)]
    lo_ap = nc.dram_tensor("logt", [C_CLS, SLOTS], F32, kind="ExternalOutput").ap()

    with tile.TileContext(nc) as tc, ExitStack() as ctx:
        cst = ctx.enter_context(tc.tile_pool(name="cst", bufs=1))

        # --- resident constants -------------------------------------------
        w1_sb = cst.tile([P, NG, KEXP], BF16)
        for g in range(NG):
            nc.sync.dma_start(out=w1_sb[:, g, :], in_=w1_ap[:, g, :])
        w2_sb = cst.tile([P, NG, KT, H], BF16)
        for g in range(NG):
            nc.sync.dma_start(out=w2_sb[:, g, :, :], in_=w2_ap[:, g, :, :])
        xt_sb = cst.tile([P, RTOT], BF16)
        for G in range(GPC):
            c0 = G * NG * K_CAP
            nc.sync.dma_start(out=xt_sb[:, c0:c0 + NG * K_CAP],
                              in_=xt_ap[:, c0:c0 + NG * K_CAP])
        b1_sb = cst.tile([P, NG * KT], F32)
        nc.sync.dma_start(out=b1_sb[:], in_=b1_ap[:])
        b2_sb = cst.tile([P, NG, H], F32)
        nc.sync.dma_start(out=b2_sb[:], in_=b2_ap[:])
        wo_sb = cst.tile([P, FT, C_CLS], F32R)
        wo_raw = cst.tile([P, FT, C_CLS], F32)
        nc.sync.dma_start(out=wo_raw[:], in_=wo_ap[:])
        nc.vector.tensor_copy(wo_sb[:], wo_raw[:])
        bo_sb = cst.tile([C_CLS, 1], F32)
        nc.sync.dma_start(out=bo_sb[:], in_=bo_ap[:])
        ic_sb = cst.tile([P, GPC], F32)
        nc.sync.dma_start(out=ic_sb[:], in_=ic_ap[:])
        gi_sb = cst.tile([P, GPC * NHALF * IW], I16)
        nc.sync.dma_start(out=gi_sb[:], in_=gi_ap[:])
        ident = cst.tile([P, P], BF16)
        nc.sync.dma_start(out=ident[:], in_=id_ap[:])

        # zero sentinel row per graph scratch (for unassigned/pad slots)
        sent0 = cst.tile([1, H], BF16)
        nc.vector.memset(sent0[:], 0.0)
        for G in range(GPC):
            nc.sync.dma_start(out=og_aps[G][ZROW:ZROW + 1, :], in_=sent0[:])

        embt = [cst.tile([P, GPC, CCAP], F32, name=f"embt{f}") for f in range(FT)]
        embn = [cst.tile([P, GPC, CCAP], F32R, name=f"embn{f}") for f in range(FT)]

        with tc.tile_pool(name="g_ht", bufs=2) as ght, \
             tc.tile_pool(name="g_og", bufs=4) as gog, \
             tc.tile_pool(name="g_gat", bufs=2) as pgat, \
             tc.tile_pool(name="g_tre", bufs=1) as ptre, \
             tc.tile_pool(name="g_emb", bufs=2) as pemb, \
             tc.tile_pool(name="g_nrm", bufs=2) as pnrm, \
             tc.tile_pool(name="g_lg", bufs=2) as plg, \
             tc.tile_pool(name="g_ph", bufs=2, space="PSUM") as gph, \
             tc.tile_pool(name="g_po", bufs=2, space="PSUM") as gpo, \
             tc.tile_pool(name="g_tp", bufs=2, space="PSUM") as gtp, \
             tc.tile_pool(name="g_pl", bufs=1, space="PSUM") as gpl:
            for G in range(GPC):
                # ---- GEMM for this graph's rows, group by group ----------
                for g in range(NG):
                    for (coff, CN) in chunks:
                        col0 = (G * NG + g) * K_CAP + coff
                        ht = ght.tile([P, KT, 512], BF16, tag="ht")
                        for kt in range(KT):
                            h_ps = gph.tile([P, 512], F32, tag="h")
                            nc.tensor.matmul(
                                h_ps[:, :CN],
                                w1_sb[:, g, kt * P:(kt + 1) * P],
                                xt_sb[:, col0:col0 + CN],
                                start=True, stop=True)
                            bsl = b1_sb[:, g * KT + kt:g * KT + kt + 1]
                            if kt % 2 == 0:
                                nc.scalar.activation(
                                    ht[:, kt, :CN], h_ps[:, :CN], AF.Relu,
                                    bias=bsl)
                            else:
                                nc.vector.tensor_scalar(
                                    ht[:, kt, :CN], h_ps[:, :CN], bsl, 0.0,
                                    op0=ALU.add, op1=ALU.max)
                        for s in range(CN // P):
                            og_ps = gpo.tile([P, H], F32, tag="og")
                            for kt in range(KT):
                                nc.tensor.matmul(
                                    og_ps[:],
                                    ht[:, kt, s * P:(s + 1) * P],
                                    w2_sb[:, g, kt, :],
                                    start=(kt == 0), stop=(kt == KT - 1))
                            og_sb = gog.tile([P, H], BF16, tag="ogsb")
                            nc.vector.tensor_tensor(
                                out=og_sb[:], in0=og_ps[:], in1=b2_sb[:, g, :],
                                op=ALU.add)
                            r0 = g * K_CAP + coff + s * P
                            nc.sync.dma_start(out=og_aps[G][r0:r0 + P, :],
                                              in_=og_sb[:])

                # ---- gather + member max, half-graph at a time -----------
                for hh in range(NHALF):
                    gat = pgat.tile([P, MCAP, H], BF16, tag="gat")
                    iw0 = (G * NHALF + hh) * IW
                    nc.gpsimd.dma_gather(
                        gat[:], og_aps[G][:], gi_sb[:, iw0:iw0 + IW],
                        MCAP * P, MCAP * P, H, single_packet=False)
                    cur = gat
                    m = MCAP
                    while m > 2:
                        m //= 2
                        nxt = ptre.tile([P, m, H], BF16, tag=f"tm{m}")
                        nc.vector.tensor_tensor(
                            out=nxt[:], in0=cur[:, 0:m, :],
                            in1=cur[:, m:2 * m, :], op=ALU.max)
                        cur = nxt
                    embh = pemb.tile([P, H], BF16, tag="embh")
                    nc.vector.tensor_tensor(
                        out=embh[:], in0=cur[:, 0:1, :].opt({0}),
                        in1=cur[:, 1:2, :].opt({0}), op=ALU.max)
                    # transpose to feature-major
                    for f in range(FT):
                        tp = gtp.tile([P, P], BF16, tag="tp")
                        nc.tensor.transpose(
                            tp[:], embh[:, f * P:(f + 1) * P], ident[:])
                        nc.scalar.activation(
                            embt[f][:, G, hh * P:(hh + 1) * P], tp[:], AF.Copy)

                # ---- instance norm + classifier for this graph -----------
                for f in range(FT):
                    et = embt[f][:, G, :]
                    sm = pnrm.tile([P, 1], F32, tag="sm")
                    nc.vector.tensor_reduce(sm[:], et, mybir.AxisListType.X,
                                            ALU.add)
                    sq = pnrm.tile([P, CCAP], F32, tag="sq")
                    nc.scalar.activation(sq[:], et, AF.Square)
                    s2 = pnrm.tile([P, 1], F32, tag="s2")
                    nc.vector.tensor_reduce(s2[:], sq[:], mybir.AxisListType.X,
                                            ALU.add)
                    mean = pnrm.tile([P, 1], F32, tag="mean")
                    nc.vector.tensor_tensor(out=mean[:], in0=sm[:],
                                            in1=ic_sb[:, G:G + 1], op=ALU.mult)
                    ex2 = pnrm.tile([P, 1], F32, tag="ex2")
                    nc.vector.tensor_tensor(out=ex2[:], in0=s2[:],
                                            in1=ic_sb[:, G:G + 1], op=ALU.mult)
                    m2 = pnrm.tile([P, 1], F32, tag="m2")
                    nc.vector.tensor_tensor(out=m2[:], in0=mean[:], in1=mean[:],
                                            op=ALU.mult)
                    var = pnrm.tile([P, 1], F32, tag="var")
                    nc.vector.tensor_tensor(out=var[:], in0=ex2[:], in1=m2[:],
                                            op=ALU.subtract)
                    ve = pnrm.tile([P, 1], F32, tag="ve")
                    nc.vector.tensor_scalar_add(ve[:], var[:], EPS)
                    sd = pnrm.tile([P, 1], F32, tag="sd")
                    nc.scalar.activation(sd[:], ve[:], AF.Sqrt)
                    rstd = pnrm.tile([P, 1], F32, tag="rstd")
                    nc.vector.reciprocal(rstd[:], sd[:])
                    nc.vector.tensor_scalar(
                        embn[f][:, G, :], et, mean[:], rstd[:],
                        op0=ALU.subtract, op1=ALU.mult)

                lg_ps = gpl.tile([C_CLS, CCAP], F32, tag="lg")
                for f in range(FT):
                    nc.tensor.matmul(lg_ps[:], wo_sb[:, f, :],
                                     embn[f][:, G, :],
                                     start=(f == 0), stop=(f == FT - 1))
                lg_sb = plg.tile([C_CLS, CCAP], F32, tag="lgs")
                nc.vector.tensor_scalar(lg_sb[:], lg_ps[:], bo_sb[:], None,
                                        op0=ALU.add)
                nc.sync.dma_start(out=lo_ap[:, G * CCAP:(G + 1) * CCAP],
                                  in_=lg_sb[:])

    nc.compile()
    return nc


# ----------------------------------------------------------------------------
# PJRT runner (reusable for timing)
# ----------------------------------------------------------------------------

class _Runner:
    def __init__(self, nc):
        from jax.sharding import Mesh, PartitionSpec
        from jax.experimental.shard_map import shard_map

        bass2jax.install_neuronx_cc_hook()
        self.nc = nc
        part_name = (nc.partition_id_tensor.name
                     if nc.partition_id_tensor else None)
        in_names, out_names, out_avals, zero_outs = [], [], [], []
        for alloc in nc.m.functions[0].allocations:
            if not isinstance(alloc, mybir.MemoryLocationSet):
                continue
            name = alloc.memorylocations[0].name
            if alloc.kind == "ExternalInput":
                if name != part_name:
                    in_names.append(name)
            elif alloc.kind == "ExternalOutput":
                out_names.append(name)
                shape = tuple(alloc.tensor_shape)
                dtype = mybir.dt.np(alloc.dtype)
                out_avals.append(jax.core.ShapedArray(shape, dtype))
                zero_outs.append(np.zeros(shape, dtype))
        self.n_params = len(in_names)
        self.in_names = in_names + out_names
        if part_name is not None:
            self.in_names = self.in_names + [part_name]
        self.out_names = out_names
        self.out_avals = out_avals
        self.zero_outs = zero_outs

        def _body(*args):
            operands = list(args)
            if part_name is not None:
                operands.append(bass2jax.partition_id_tensor())
            outs = bass2jax._bass_exec_p.bind(
                *operands,
                out_avals=tuple(out_avals),
                in_names=tuple(self.in_names),
                out_names=tuple(out_names),
                lowering_input_output_aliases=(),
                sim_require_finite=True,
                sim_require_nnan=True,
                nc=nc,
            )
            return tuple(outs)

        devices = jax.devices()[:NCORES]
        self.mesh = Mesh(np.asarray(devices), ("core",))
        n_all = self.n_params + len(out_names)
        self.fn = jax.jit(
            shard_map(_body, mesh=self.mesh,
                      in_specs=(PartitionSpec("core"),) * n_all,
                      out_specs=(PartitionSpec("core"),) * len(out_names),
                      check_rep=False),
            keep_unused=True,
        )

    def prepare(self, in_maps):
        concat = [
            np.concatenate([np.asarray(m[nm]) for m in in_maps], axis=0)
            for nm in self.in_names[:self.n_params]
        ]
        concat += [
            np.zeros((NCORES * z.shape[0], *z.shape[1:]), z.dtype)
            for z in self.zero_outs
        ]
        return concat

    def run(self, args):
        outs = self.fn(*args)
        return [
            {nm: np.asarray(outs[i]).reshape(NCORES, *self.out_avals[i].shape)[c]
             for i, nm in enumerate(self.out_names)}
            for c in range(NCORES)
        ]


# ----------------------------------------------------------------------------
# Host-side sharding / index plumbing
# ----------------------------------------------------------------------------

def _round_up(v, m):
    return (v + m - 1) // m * m


def _pow2_round(v):
    p = 1
    while p < v:
        p *= 2
    return p


def prepare(x, group_indices, pool_cluster_fine, batch_cluster_coarse,
            W1, b1, W2, b2, w_out, b_out):
    """Compute capacities + per-core input maps. Returns (key, in_maps, meta)."""
    x = np.asarray(x)
    gidx = np.asarray(group_indices)
    pcf = np.asarray(pool_cluster_fine).astype(np.int64)
    bcc = np.asarray(batch_cluster_coarse).astype(np.int64)
    W1 = np.asarray(W1, dtype=np.float32)
    b1 = np.asarray(b1, dtype=np.float32)
    W2 = np.asarray(W2, dtype=np.float32)
    b2 = np.asarray(b2, dtype=np.float32)
    w_out = np.asarray(w_out, dtype=np.float32)
    b_out = np.asarray(b_out, dtype=np.float32)

    GPC = G_SEG // NCORES

    # node -> group (later groups win on duplicates, matching scatter order)
    gid = np.full(N, -1, np.int32)
    for g in range(NG):
        gid[gidx[g]] = g

    # graph boundaries in cluster space; cluster boundaries in node space
    g_lo = np.searchsorted(bcc, np.arange(G_SEG))
    g_hi = np.searchsorted(bcc, np.arange(G_SEG), "right")
    g_sz = g_hi - g_lo
    CCAP = _round_up(max(1, int(g_sz.max())), P)
    cl_lo = np.searchsorted(pcf, np.arange(F_SEG))
    cl_hi = np.searchsorted(pcf, np.arange(F_SEG), "right")
    cl_sz = cl_hi - cl_lo
    MCAP = _pow2_round(max(2, int(cl_sz.max())))
    NHALF = CCAP // P
    IW = MCAP * P // 16

    # rows per (core, graph, group)
    counts = np.zeros((NCORES, GPC, NG), np.int64)
    for c in range(NCORES):
        for gi_ in range(GPC):
            gg = c * GPC + gi_
            nd_lo, nd_hi = int(cl_lo[g_lo[gg]]) if g_sz[gg] else 0, 0
            if g_sz[gg]:
                nd_hi = int(cl_hi[g_hi[gg] - 1])
            nd = np.arange(nd_lo, nd_hi)
            gs = gid[nd]
            for g in range(NG):
                counts[c, gi_, g] = int((gs == g).sum())
    K_CAP = _round_up(max(1, int(counts.max())), P)
    RTOT = GPC * NG * K_CAP
    ZROW = NG * K_CAP
    assert ZROW + 1 < 32768, f"K_CAP={K_CAP} too large for int16 gather indices"
    SLOTS = GPC * CCAP

    # replicated weight prep (shared across cores)
    w1_h = np.ascontiguousarray(W1.transpose(1, 0, 2)).astype(NPBF16)
    w2_h = np.ascontiguousarray(
        W2.reshape(NG, KT, P, H).transpose(2, 0, 1, 3)).astype(NPBF16)
    b1_h = np.ascontiguousarray(
        b1.reshape(NG, KT, P).transpose(2, 0, 1).reshape(P, -1))
    b2_h = np.ascontiguousarray(
        np.broadcast_to(b2[None, :, :], (P, NG, H))).astype(np.float32)
    wo_h = np.ascontiguousarray(
        w_out.reshape(FT, P, C_CLS).transpose(1, 0, 2))
    bo_h = np.ascontiguousarray(b_out.reshape(C_CLS, 1))
    id_h = np.eye(P, dtype=np.float32).astype(NPBF16)

    x_bf = x.astype(NPBF16)

    in_maps = []
    meta = []
    for c in range(NCORES):
        xt = np.zeros((P, RTOT), NPBF16)
        rowof = {}           # (graph-local) node -> og row, per graph
        inv_cnt = np.zeros(GPC, np.float32)
        gidx_w = np.zeros((P, GPC * NHALF * IW), np.int16)
        core_graphs = []
        for gi_ in range(GPC):
            gg = c * GPC + gi_
            n_cl = int(g_sz[gg])
            inv_cnt[gi_] = 1.0 / max(n_cl, 1)
            clusters = np.arange(g_lo[gg], g_hi[gg])
            core_graphs.append(clusters)
            if n_cl == 0:
                continue
            nd_lo, nd_hi = int(cl_lo[clusters[0]]), int(cl_hi[clusters[-1]])
            nd = np.arange(nd_lo, nd_hi)
            gs = gid[nd]
            rows = np.full(nd_hi - nd_lo, ZROW, np.int32)
            for g in range(NG):
                sel = nd[gs == g]
                cnt = len(sel)
                col0 = (gi_ * NG + g) * K_CAP
                xt[:, col0:col0 + cnt] = x_bf[sel].T
                rows[sel - nd_lo] = g * K_CAP + np.arange(cnt, dtype=np.int32)

            # member slot table for this graph: [CCAP, MCAP]
            member = np.full((CCAP, MCAP), ZROW, np.int32)
            for j, f in enumerate(clusters):
                sz = int(cl_sz[f])
                if sz == 0:
                    continue
                mr = rows[int(cl_lo[f]) - nd_lo:int(cl_hi[f]) - nd_lo]
                member[j, :sz] = mr[:MCAP] if sz > MCAP else mr
                if sz < MCAP:
                    member[j, sz:] = mr[0]
            for hh in range(NHALF):
                mt = member[hh * P:(hh + 1) * P]          # [128, MCAP]
                seq = mt.T.reshape(-1)                     # i = m*128 + cpos
                w = seq.reshape(-1, 16).T.astype(np.int16)
                iw0 = (gi_ * NHALF + hh) * IW
                gidx_w[:, iw0:iw0 + IW] = np.tile(w, (8, 1))

        in_maps.append({
            "xt": xt,
            "w1b": w1_h, "w2b": w2_h, "b1s": b1_h, "b2r": b2_h,
            "wout": wo_h, "bout": bo_h,
            "invc": np.broadcast_to(inv_cnt[None, :], (P, GPC)).copy(),
            "ident": id_h,
            "gidx": gidx_w,
        })
        meta.append({"graphs": core_graphs, "c": c})

    key = (K_CAP, CCAP, MCAP)
    return key, in_maps, meta, (CCAP,)


def get_runner(key):
    if key not in _PROGRAM_CACHE:
        nc = _build_program(*key)
        _PROGRAM_CACHE[key] = _Runner(nc)
    return _PROGRAM_CACHE[key]


def kernel(**inputs) -> np.ndarray:
    key, in_maps, meta, (CCAP,) = prepare(**inputs)
    runner = get_runner(key)
    args = runner.prepare(in_maps)
    results = runner.run(args)

    bcc = np.asarray(inputs["batch_cluster_coarse"]).astype(np.int64)
    GPC = G_SEG // NCORES
    g_lo = np.searchsorted(bcc, np.arange(G_SEG))
    out = np.zeros((F_SEG, C_CLS), np.float32)
    for c in range(NCORES):
        lo = results[c]["logt"]              # [16, SLOTS]
        for gi_, clusters in enumerate(meta[c]["graphs"]):
            for f in clusters:
                slot = gi_ * CCAP + (int(f) - int(g_lo[c * GPC + gi_]))
                out[f] = lo[:, slot]
    return out


# revision 21
# speedup vs baseline: 1.0277x; 1.0277x over previous
"""Trainium2 Bass kernel for nn_ClusterModel (MoE routing + segment pooling).

Model:
  xg = x[group_indices]                         # [4, N/4, 128] per-group gather
  h  = relu(xg @ W1[g] + b1[g])                 # [4, N/4, 1024]
  og = h @ W2[g] + b2[g]                        # [4, N/4, 512]
  new_feat = scatter(og) back to node order     # [N, 512]
  emb = segment_max(new_feat, fine clusters)    # [8192, 512]  (16 nodes/cluster)
  normed = InstanceNorm per coarse graph        # [8192, 512]  (256 clusters/graph)
  logits = normed @ w_out + b_out               # [8192, 16]

Sharding: 8 cores, each takes 4 consecutive coarse graphs (= 16384 nodes =
1024 fine clusters).  All segment reductions are core-local -> no collectives.

Per-core pipeline (graph-major, all engines overlapped):
  for G in 4 graphs:
    for each 512-row chunk index, for g in 4 groups:  bf16 GEMM
      L1 (W1 stationary, xt moving)  -> h feature-major in PSUM
      relu+b1 (ACT/DVE alternating)  -> ht bf16
      L2 (ht stationary, W2 moving)  -> og row-major [128 rows, 512] in PSUM
      +b2 (DVE) -> og bf16 -> DMA to per-graph DRAM scratch
      ... and as soon as a half-graph's referenced rows are all written
      (HJ, host-computed), fire its 2 member-split dma_gathers (gpsimd,
      non-transpose, 1KB row descriptors) -> [128cl, 8m, 512] x2
    pool(G) is emitted after GEMM(G+1) with a tile_wait_until floor so the
    strict per-engine FIFOs never head-of-line block on the gather:
      DVE in-place pairwise max tree over members -> emb half [128, 512] bf16
      PE transpose -> feature-major embT; InstanceNorm (in-place); classifier.

GEMM inputs bf16 (full PE rate, FWL weight loads); stats/classifier f32.
Gathers/pool of graph G overlap graph G+1's GEMM; PE stays HAM-warm.
"""

import numpy as np
from contextlib import ExitStack

import jax
import concourse.bass as bass
import concourse.tile as tile
from concourse import bacc, mybir
from concourse import bass2jax

F32 = mybir.dt.float32
F32R = mybir.dt.float32r
BF16 = mybir.dt.bfloat16
I16 = mybir.dt.int16
AF = mybir.ActivationFunctionType
ALU = mybir.AluOpType
NPBF16 = mybir.dt.np(BF16)

# Problem constants (hardcoded per contest contract)
N = 131072
D = 128
KEXP = 1024
H = 512
NG = 4
F_SEG = 8192
G_SEG = 32
C_CLS = 16
EPS = 1e-5
NCORES = 8
P = 128
KT = KEXP // P                 # 8 k-tiles in layer 2
FT = H // P                    # 4 feature tiles
NEG = -3.0e38

_PROGRAM_CACHE: dict = {}


# ----------------------------------------------------------------------------
# Device program
# ----------------------------------------------------------------------------

def _build_program(K_CAP: int, CCAP: int, MCAP: int, HJ: tuple = ()):
    """SPMD program. K_CAP: padded rows per (graph, group), multiple of 128.
    CCAP: padded clusters per graph, multiple of 128. MCAP: padded members
    per cluster, power of two. HJ[hh]: number of GEMM chunks after which the
    half-graph-hh gather may fire (max rows referenced, host-computed)."""
    GPC = G_SEG // NCORES          # graphs per core = 4
    NHALF = CCAP // P              # gather blocks per graph (128 clusters each)
    RTOT = GPC * NG * K_CAP        # xt columns per core
    RT_G = NG * K_CAP + 1          # og rows per graph (+1 zero row)
    ZROW = NG * K_CAP
    SLOTS = GPC * CCAP
    IW = MCAP * P // 16            # idx words per gather block

    # chunk split of K_CAP into <=512 pieces (multiples of 128)
    chunks = []
    off = 0
    while off < K_CAP:
        n = min(512, K_CAP - off)
        chunks.append((off, n))
        off += n
    if not HJ:
        HJ = tuple(len(chunks) for _ in range(NHALF))
    assert len(HJ) == NHALF and all(1 <= j <= len(chunks) for j in HJ)

    nc = bacc.Bacc("TRN2", target_bir_lowering=False, debug=False,
                   num_devices=NCORES)

    xt_ap = nc.dram_tensor("xt", [P, RTOT], BF16, kind="ExternalInput").ap()
    w1_ap = nc.dram_tensor("w1b", [P, NG, KEXP], BF16, kind="ExternalInput").ap()
    w2_ap = nc.dram_tensor("w2b", [P, NG, KT, H], BF16, kind="ExternalInput").ap()
    b1_ap = nc.dram_tensor("b1s", [P, NG * KT], F32, kind="ExternalInput").ap()
    b2_ap = nc.dram_tensor("b2r", [P, NG, H], F32, kind="ExternalInput").ap()
    wo_ap = nc.dram_tensor("wout", [P, FT, C_CLS], F32, kind="ExternalInput").ap()
    bo_ap = nc.dram_tensor("bout", [C_CLS, 1], F32, kind="ExternalInput").ap()
    ic_ap = nc.dram_tensor("invc", [P, GPC], F32, kind="ExternalInput").ap()
    gi_ap = nc.dram_tensor("gidx", [P, GPC * NHALF * IW], I16,
                           kind="ExternalInput").ap()
    id_ap = nc.dram_tensor("ident", [P, P], BF16, kind="ExternalInput").ap()
    og_aps = [nc.dram_tensor(f"ogs{G}", [RT_G, H], BF16).ap()
              for G in range(GPC)]
    lo_ap = nc.dram_tensor("logt", [C_CLS, SLOTS], F32, kind="ExternalOutput").ap()

    with tile.TileContext(nc) as tc, ExitStack() as ctx:
        cst = ctx.enter_context(tc.tile_pool(name="cst", bufs=1))

        # --- resident constants -------------------------------------------
        w1_sb = cst.tile([P, NG, KEXP], BF16)
        for g in range(NG):
            nc.sync.dma_start(out=w1_sb[:, g, :], in_=w1_ap[:, g, :])
        xt_sb = cst.tile([P, RTOT], BF16)
        w2_sb = cst.tile([P, NG, KT, H], BF16)
        nc.sync.dma_start(out=xt_sb[:, 0:NG * K_CAP],
                          in_=xt_ap[:, 0:NG * K_CAP])
        b1_sb = cst.tile([P, NG * KT], F32)
        nc.sync.dma_start(out=b1_sb[:], in_=b1_ap[:])
        b2_sb = cst.tile([P, NG, H], F32)
        nc.sync.dma_start(out=b2_sb[:], in_=b2_ap[:])
        for g in range(NG):
            nc.sync.dma_start(out=w2_sb[:, g, :, :], in_=w2_ap[:, g, :, :])
        for G in range(1, GPC):
            c0 = G * NG * K_CAP
            nc.sync.dma_start(out=xt_sb[:, c0:c0 + NG * K_CAP],
                              in_=xt_ap[:, c0:c0 + NG * K_CAP])
        wo_sb = cst.tile([P, FT, C_CLS], F32)
        nc.sync.dma_start(out=wo_sb[:], in_=wo_ap[:])
        bo_sb = cst.tile([C_CLS, 1], F32)
        nc.sync.dma_start(out=bo_sb[:], in_=bo_ap[:])
        ic_sb = cst.tile([P, GPC], F32)
        nc.sync.dma_start(out=ic_sb[:], in_=ic_ap[:])
        gi_sb = cst.tile([P, GPC * NHALF * IW], I16)
        nc.sync.dma_start(out=gi_sb[:], in_=gi_ap[:])
        ident = cst.tile([P, P], BF16)
        nc.sync.dma_start(out=ident[:], in_=id_ap[:])

        # zero sentinel row per graph scratch (for unassigned/pad slots)
        sent0 = cst.tile([1, H], BF16)
        nc.vector.memset(sent0[:], 0.0)
        for G in range(GPC):
            nc.sync.dma_start(out=og_aps[G][ZROW:ZROW + 1, :], in_=sent0[:])

        embt = [cst.tile([P, GPC, CCAP], F32, name=f"embt{f}") for f in range(FT)]

        with tc.tile_pool(name="g_ht", bufs=2) as ght, \
             tc.tile_pool(name="g_og", bufs=20) as gog, \
             tc.tile_pool(name="g_gat", bufs=6) as pgat, \
             tc.tile_pool(name="g_emb", bufs=2) as pemb, \
             tc.tile_pool(name="g_nrm", bufs=2) as pnrm, \
             tc.tile_pool(name="g_lg", bufs=2) as plg, \
             tc.tile_pool(name="g_ph", bufs=3, space="PSUM") as gph, \
             tc.tile_pool(name="g_po", bufs=3, space="PSUM") as gpo, \
             tc.tile_pool(name="g_tp", bufs=1, space="PSUM") as gtp, \
             tc.tile_pool(name="g_pl", bufs=1, space="PSUM") as gpl:

            ACT_KT = (0, 1, 2, 4, 5, 6)    # relu engine split: 6 ACT / 2 DVE

            def emit_gemm(G):
                for j, (coff, CN) in enumerate(chunks):
                    for g in range(NG):
                        col0 = (G * NG + g) * K_CAP + coff
                        ht = ght.tile([P, KT, 512], BF16, tag="ht")
                        for kt in range(KT):
                            h_ps = gph.tile([P, 512], F32, tag="h")
                            nc.tensor.matmul(
                                h_ps[:, :CN],
                                w1_sb[:, g, kt * P:(kt + 1) * P],
                                xt_sb[:, col0:col0 + CN],
                                start=True, stop=True)
                            bsl = b1_sb[:, g * KT + kt:g * KT + kt + 1]
                            if kt in ACT_KT:
                                nc.scalar.activation(
                                    ht[:, kt, :CN], h_ps[:, :CN], AF.Relu,
                                    bias=bsl)
                            else:
                                nc.vector.tensor_scalar(
                                    ht[:, kt, :CN], h_ps[:, :CN], bsl, 0.0,
                                    op0=ALU.add, op1=ALU.max)
                        for s in range(CN // P):
                            og_ps = gpo.tile([P, H], F32, tag="og")
                            for kt in range(KT):
                                nc.tensor.matmul(
                                    og_ps[:],
                                    ht[:, kt, s * P:(s + 1) * P],
                                    w2_sb[:, g, kt, :],
                                    start=(kt == 0), stop=(kt == KT - 1))
                            og_sb = gog.tile([P, H], BF16, tag="ogsb")
                            nc.vector.tensor_tensor(
                                out=og_sb[:], in0=og_ps[:], in1=b2_sb[:, g, :],
                                op=ALU.add)
                            r0 = g * K_CAP + coff + s * P
                            nc.sync.dma_start(out=og_aps[G][r0:r0 + P, :],
                                              in_=og_sb[:])
                    # fire a half-graph gather as soon as every row it
                    # references has been written (emission-ordered deps)
                    for hh in range(NHALF):
                        if HJ[hh] == j + 1:
                            emit_gather_half(G, hh)

            gats = {}

            MH = MCAP // 2
            IWH = IW // 2

            def emit_gather_half(G, hh):
                iw0 = (G * NHALF + hh) * IW
                pair = []
                for mh in range(2):
                    gat = pgat.tile([P, MH, H], BF16, tag="gat")
                    nc.gpsimd.dma_gather(
                        gat[:], og_aps[G][:],
                        gi_sb[:, iw0 + mh * IWH:iw0 + (mh + 1) * IWH],
                        MH * P, MH * P, H, single_packet=False)
                    pair.append(gat)
                gats[(G, hh)] = pair

            def emit_pool(G):
                # member max tree + transpose to feature-major
                for hh in range(NHALF):
                    ga, gb = gats.pop((G, hh))
                    for gat in (ga, gb):
                        m = MH
                        while m > 1:
                            m //= 2
                            # in-place pairwise max halving in the gather tile
                            nc.vector.tensor_tensor(
                                out=gat[:, 0:m, :], in0=gat[:, 0:m, :],
                                in1=gat[:, m:2 * m, :], op=ALU.max)
                    embh = pemb.tile([P, H], BF16, tag="embh")
                    nc.vector.tensor_tensor(
                        out=embh[:], in0=ga[:, 0:1, :].opt({0}),
                        in1=gb[:, 0:1, :].opt({0}), op=ALU.max)
                    for f in range(FT):
                        tp = gtp.tile([P, P], BF16, tag="tp")
                        nc.tensor.transpose(
                            tp[:], embh[:, f * P:(f + 1) * P], ident[:])
                        nc.scalar.activation(
                            embt[f][:, G, hh * P:(hh + 1) * P], tp[:], AF.Copy)

                # instance norm (in-place on embt) + classifier
                for f in range(FT):
                    et = embt[f][:, G, :]
                    sm = pnrm.tile([P, 1], F32, tag="sm")
                    nc.vector.tensor_reduce(sm[:], et, mybir.AxisListType.X,
                                            ALU.add)
                    sq = pnrm.tile([P, CCAP], F32, tag="sq")
                    nc.scalar.activation(sq[:], et, AF.Square)
                    s2 = pnrm.tile([P, 1], F32, tag="s2")
                    nc.vector.tensor_reduce(s2[:], sq[:], mybir.AxisListType.X,
                                            ALU.add)
                    mean = pnrm.tile([P, 1], F32, tag="mean")
                    nc.vector.tensor_tensor(out=mean[:], in0=sm[:],
                                            in1=ic_sb[:, G:G + 1], op=ALU.mult)
                    ex2 = pnrm.tile([P, 1], F32, tag="ex2")
                    nc.vector.tensor_tensor(out=ex2[:], in0=s2[:],
                                            in1=ic_sb[:, G:G + 1], op=ALU.mult)
                    m2 = pnrm.tile([P, 1], F32, tag="m2")
                    nc.vector.tensor_tensor(out=m2[:], in0=mean[:], in1=mean[:],
                                            op=ALU.mult)
                    var = pnrm.tile([P, 1], F32, tag="var")
                    nc.vector.tensor_tensor(out=var[:], in0=ex2[:], in1=m2[:],
                                            op=ALU.subtract)
                    ve = pnrm.tile([P, 1], F32, tag="ve")
                    nc.vector.tensor_scalar_add(ve[:], var[:], EPS)
                    sd = pnrm.tile([P, 1], F32, tag="sd")
                    nc.scalar.activation(sd[:], ve[:], AF.Sqrt)
                    rstd = pnrm.tile([P, 1], F32, tag="rstd")
                    nc.vector.reciprocal(rstd[:], sd[:])
                    nc.vector.tensor_scalar(
                        et, et, mean[:], rstd[:],
                        op0=ALU.subtract, op1=ALU.mult)

                lg_ps = gpl.tile([C_CLS, CCAP], F32, tag="lg")
                for f in range(FT):
                    nc.tensor.matmul(lg_ps[:], wo_sb[:, f, :],
                                     embt[f][:, G, :],
                                     start=(f == 0), stop=(f == FT - 1))
                lg_sb = plg.tile([C_CLS, CCAP], F32, tag="lgs")
                nc.vector.tensor_scalar(lg_sb[:], lg_ps[:], bo_sb[:], None,
                                        op0=ALU.add)
                nc.sync.dma_start(out=lo_ap[:, G * CCAP:(G + 1) * CCAP],
                                  in_=lg_sb[:])

            # graph-major software pipeline: graph G's pool/norm work must
            # land mid-way through graph G+1's GEMM in every engine stream,
            # else the strict per-engine FIFOs head-of-line block on the
            # ~35us gather at each graph boundary.  The Tile scheduler's
            # cost model underestimates the gather, so pin pool(G) with a
            # virtual-time floor instead of relying on emission order.
            TS_MS = (16 * K_CAP * 40 / 2.4) * 1.35e-6 / GPC   # sim ms/graph
            for G in range(GPC):
                emit_gemm(G)
                if G >= 1:
                    with tc.tile_wait_until((G + 0.5) * TS_MS + 0.06):
                        emit_pool(G - 1)
            with tc.tile_wait_until((GPC + 0.5) * TS_MS + 0.06):
                emit_pool(GPC - 1)

    nc.compile()
    return nc


# ----------------------------------------------------------------------------
# PJRT runner (reusable for timing)
# ----------------------------------------------------------------------------

class _Runner:
    def __init__(self, nc):
        from jax.sharding import Mesh, PartitionSpec
        from jax.experimental.shard_map import shard_map

        bass2jax.install_neuronx_cc_hook()
        self.nc = nc
        part_name = (nc.partition_id_tensor.name
                     if nc.partition_id_tensor else None)
        in_names, out_names, out_avals, zero_outs = [], [], [], []
        for alloc in nc.m.functions[0].allocations:
            if not isinstance(alloc, mybir.MemoryLocationSet):
                continue
            name = alloc.memorylocations[0].name
            if alloc.kind == "ExternalInput":
                if name != part_name:
                    in_names.append(name)
            elif alloc.kind == "ExternalOutput":
                out_names.append(name)
                shape = tuple(alloc.tensor_shape)
                dtype = mybir.dt.np(alloc.dtype)
                out_avals.append(jax.core.ShapedArray(shape, dtype))
                zero_outs.append(np.zeros(shape, dtype))
        self.n_params = len(in_names)
        self.in_names = in_names + out_names
        if part_name is not None:
            self.in_names = self.in_names + [part_name]
        self.out_names = out_names
        self.out_avals = out_avals
        self.zero_outs = zero_outs

        def _body(*args):
            operands = list(args)
            if part_name is not None:
                operands.append(bass2jax.partition_id_tensor())
            outs = bass2jax._bass_exec_p.bind(
                *operands,
                out_avals=tuple(out_avals),
                in_names=tuple(self.in_names),
                out_names=tuple(out_names),
                lowering_input_output_aliases=(),
                sim_require_finite=True,
                sim_require_nnan=True,
                nc=nc,
            )
            return tuple(outs)

        devices = jax.devices()[:NCORES]
        self.mesh = Mesh(np.asarray(devices), ("core",))
        n_all = self.n_params + len(out_names)
        self.fn = jax.jit(
            shard_map(_body, mesh=self.mesh,
                      in_specs=(PartitionSpec("core"),) * n_all,
                      out_specs=(PartitionSpec("core"),) * len(out_names),
                      check_rep=False),
            keep_unused=True,
        )

    def prepare(self, in_maps):
        concat = [
            np.concatenate([np.asarray(m[nm]) for m in in_maps], axis=0)
            for nm in self.in_names[:self.n_params]
        ]
        concat += [
            np.zeros((NCORES * z.shape[0], *z.shape[1:]), z.dtype)
            for z in self.zero_outs
        ]
        return concat

    def run(self, args):
        outs = self.fn(*args)
        return [
            {nm: np.asarray(outs[i]).reshape(NCORES, *self.out_avals[i].shape)[c]
             for i, nm in enumerate(self.out_names)}
            for c in range(NCORES)
        ]


# ----------------------------------------------------------------------------
# Host-side sharding / index plumbing
# ----------------------------------------------------------------------------

def _round_up(v, m):
    return (v + m - 1) // m * m


def _pow2_round(v):
    p = 1
    while p < v:
        p *= 2
    return p


def prepare(x, group_indices, pool_cluster_fine, batch_cluster_coarse,
            W1, b1, W2, b2, w_out, b_out):
    """Compute capacities + per-core input maps. Returns (key, in_maps, meta)."""
    x = np.asarray(x)
    gidx = np.asarray(group_indices)
    pcf = np.asarray(pool_cluster_fine).astype(np.int64)
    bcc = np.asarray(batch_cluster_coarse).astype(np.int64)
    W1 = np.asarray(W1, dtype=np.float32)
    b1 = np.asarray(b1, dtype=np.float32)
    W2 = np.asarray(W2, dtype=np.float32)
    b2 = np.asarray(b2, dtype=np.float32)
    w_out = np.asarray(w_out, dtype=np.float32)
    b_out = np.asarray(b_out, dtype=np.float32)

    GPC = G_SEG // NCORES

    # node -> group (later groups win on duplicates, matching scatter order)
    gid = np.full(N, -1, np.int32)
    for g in range(NG):
        gid[gidx[g]] = g

    # graph boundaries in cluster space; cluster boundaries in node space
    g_lo = np.searchsorted(bcc, np.arange(G_SEG))
    g_hi = np.searchsorted(bcc, np.arange(G_SEG), "right")
    g_sz = g_hi - g_lo
    CCAP = _round_up(max(1, int(g_sz.max())), P)
    cl_lo = np.searchsorted(pcf, np.arange(F_SEG))
    cl_hi = np.searchsorted(pcf, np.arange(F_SEG), "right")
    cl_sz = cl_hi - cl_lo
    MCAP = _pow2_round(max(2, int(cl_sz.max())))
    NHALF = CCAP // P
    IW = MCAP * P // 16

    # rows per (core, graph, group)
    counts = np.zeros((NCORES, GPC, NG), np.int64)
    for c in range(NCORES):
        for gi_ in range(GPC):
            gg = c * GPC + gi_
            nd_lo, nd_hi = int(cl_lo[g_lo[gg]]) if g_sz[gg] else 0, 0
            if g_sz[gg]:
                nd_hi = int(cl_hi[g_hi[gg] - 1])
            nd = np.arange(nd_lo, nd_hi)
            gs = gid[nd]
            for g in range(NG):
                counts[c, gi_, g] = int((gs == g).sum())
    K_CAP = _round_up(max(1, int(counts.max())), P)
    RTOT = GPC * NG * K_CAP
    ZROW = NG * K_CAP
    assert ZROW + 1 < 32768, f"K_CAP={K_CAP} too large for int16 gather indices"
    SLOTS = GPC * CCAP

    # replicated weight prep (shared across cores)
    w1_h = np.ascontiguousarray(W1.transpose(1, 0, 2)).astype(NPBF16)
    w2_h = np.ascontiguousarray(
        W2.reshape(NG, KT, P, H).transpose(2, 0, 1, 3)).astype(NPBF16)
    b1_h = np.ascontiguousarray(
        b1.reshape(NG, KT, P).transpose(2, 0, 1).reshape(P, -1))
    b2_h = np.ascontiguousarray(
        np.broadcast_to(b2[None, :, :], (P, NG, H))).astype(np.float32)
    wo_h = np.ascontiguousarray(
        w_out.reshape(FT, P, C_CLS).transpose(1, 0, 2))
    bo_h = np.ascontiguousarray(b_out.reshape(C_CLS, 1))
    id_h = np.eye(P, dtype=np.float32).astype(NPBF16)

    x_bf = x.astype(NPBF16)

    need = np.zeros(NHALF, np.int64)      # rows referenced per half-gather
    in_maps = []
    meta = []
    for c in range(NCORES):
        xt = np.zeros((P, RTOT), NPBF16)
        rowof = {}           # (graph-local) node -> og row, per graph
        inv_cnt = np.zeros(GPC, np.float32)
        gidx_w = np.zeros((P, GPC * NHALF * IW), np.int16)
        core_graphs = []
        for gi_ in range(GPC):
            gg = c * GPC + gi_
            n_cl = int(g_sz[gg])
            inv_cnt[gi_] = 1.0 / max(n_cl, 1)
            clusters = np.arange(g_lo[gg], g_hi[gg])
            core_graphs.append(clusters)
            if n_cl == 0:
                continue
            nd_lo, nd_hi = int(cl_lo[clusters[0]]), int(cl_hi[clusters[-1]])
            nd = np.arange(nd_lo, nd_hi)
            gs = gid[nd]
            rows = np.full(nd_hi - nd_lo, ZROW, np.int32)
            for g in range(NG):
                sel = nd[gs == g]
                cnt = len(sel)
                col0 = (gi_ * NG + g) * K_CAP
                xt[:, col0:col0 + cnt] = x_bf[sel].T
                rows[sel - nd_lo] = g * K_CAP + np.arange(cnt, dtype=np.int32)

            # member slot table for this graph: [CCAP, MCAP]
            member = np.full((CCAP, MCAP), ZROW, np.int32)
            for j, f in enumerate(clusters):
                sz = int(cl_sz[f])
                if sz == 0:
                    continue
                mr = rows[int(cl_lo[f]) - nd_lo:int(cl_hi[f]) - nd_lo]
                member[j, :sz] = mr[:MCAP] if sz > MCAP else mr
                if sz < MCAP:
                    member[j, sz:] = mr[0]
            for hh in range(NHALF):
                mt = member[hh * P:(hh + 1) * P]          # [128, MCAP]
                valid = mt[mt != ZROW]
                if valid.size:
                    need[hh] = max(need[hh], int((valid % K_CAP).max()) + 1)
                seq = mt.T.reshape(-1)                     # i = m*128 + cpos
                w = seq.reshape(-1, 16).T.astype(np.int16)
                iw0 = (gi_ * NHALF + hh) * IW
                gidx_w[:, iw0:iw0 + IW] = np.tile(w, (8, 1))

        in_maps.append({
            "xt": xt,
            "w1b": w1_h, "w2b": w2_h, "b1s": b1_h, "b2r": b2_h,
            "wout": wo_h, "bout": bo_h,
            "invc": np.broadcast_to(inv_cnt[None, :], (P, GPC)).copy(),
            "ident": id_h,
            "gidx": gidx_w,
        })
        meta.append({"graphs": core_graphs, "c": c})

    # chunk index after which each half-graph gather may fire
    cum, HJ = 0, []
    csz = []
    off = 0
    while off < K_CAP:
        n = min(512, K_CAP - off)
        csz.append(n)
        off += n
    for hh in range(NHALF):
        cum, j = 0, 0
        while j < len(csz) and cum < need[hh]:
            cum += csz[j]
            j += 1
        HJ.append(max(1, j))
    key = (K_CAP, CCAP, MCAP, tuple(HJ))
    return key, in_maps, meta, (CCAP,)


def get_runner(key):
    if key not in _PROGRAM_CACHE:
        nc = _build_program(*key)
        _PROGRAM_CACHE[key] = _Runner(nc)
    return _PROGRAM_CACHE[key]


def kernel(**inputs) -> np.ndarray:
    key, in_maps, meta, (CCAP,) = prepare(**inputs)
    runner = get_runner(key)
    args = runner.prepare(in_maps)
    results = runner.run(args)

    bcc = np.asarray(inputs["batch_cluster_coarse"]).astype(np.int64)
    GPC = G_SEG // NCORES
    g_lo = np.searchsorted(bcc, np.arange(G_SEG))
    out = np.zeros((F_SEG, C_CLS), np.float32)
    for c in range(NCORES):
        lo = results[c]["logt"]              # [16, SLOTS]
        for gi_, clusters in enumerate(meta[c]["graphs"]):
            for f in clusters:
                slot = gi_ * CCAP + (int(f) - int(g_lo[c * GPC + gi_]))
                out[f] = lo[:, slot]
    return out
